# revision 7
# baseline (speedup 1.0000x reference)
"""Trainium2 Bass kernel for GNN message passing (nn_FALR2_35794257445089).

Math (per batch element b, all shapes per-core):
    z = concat(node_fts, hidden)                       (n, 2h)
    msgs[i, j, m] = msg1[j,m] + msg2[i,m] + msgE[i,j,m] + msgG[m]
    out_msgs[j, m] = max_i msgs[i,j,m] * adj[i,j]
    ret = z @ W_o1 + b_o1 + out_msgs @ W_o2 + b_o2

Strategy: data-parallel over b across 8 cores. On each core the dominant
work is msgE = edge_fts[b] @ W_me (32 MiB of edge data streamed once) and
the masked max over i.

Masking is done additively: x_i = msgE_i + c_i + M'_i with
M'[i,j] = (adj[i,j]-1)*1e9 (0 when adj=1, -1e9 when adj=0), which makes
the i-independent term msg1 pull out of the max entirely. The final
result is max(acc + msg1T, zb[j]) where zb[j] = 0 if column j has any
adj==0 (the reference's "masked entries contribute 0 to the max"), else
-1e30, and all-zero columns come out exactly 0.

Per-core inner loop (i = source node, 256 iters):
  PE : 2 transposes of edge[i] (j,h)->(h,j), 1 matmul W_me^T @ edgeT
       (N=256), 1 K=2 rank-1 adding c_i (x) ones + ones (x) M'_i
  ACT: copy transposed-edge PSUM->SBUF (the matmul input)
  DVE: one grouped tensor_reduce(max) per 5 planes, chained through the
       previous group's accumulator slot (ping-pong)
Small i-independent terms (msg1T, z@W_o1+biases, c = z@W_m2 + graph-term,
M', zb, table packing) are precomputed on the host - they are O(n*h^2)
vs the O(n^2*h^2) device work.
"""

import os
import sys

import numpy as np

if "/opt/trn_rl_repo" not in sys.path:
    sys.path.insert(0, "/opt/trn_rl_repo")

import concourse.bass as bass
import concourse.bacc as bacc
import concourse.mybir as mybir
import concourse.tile as tile
from concourse.tile import add_dep_helper
from concourse.bass_utils import run_bass_kernel_spmd

B, N, H, MID, OUT = 8, 256, 128, 128, 128
F32 = mybir.dt.float32
BIG = 1.0e9
NEG = -1.0e30

GROUPS = [5] * 51 + [1]  # 256 source nodes
assert sum(GROUPS) == N


def build_nc():
    nc = bacc.Bacc("TRN2", target_bir_lowering=False, debug=False)

    edge = nc.dram_tensor("edge", [N, N, H], F32, kind="ExternalInput")
    ctab_d = nc.dram_tensor("ctab", [8, 64 * MID], F32, kind="ExternalInput")
    mtab_d = nc.dram_tensor("mtab", [8, 64 * N], F32, kind="ExternalInput")
    msg1t_d = nc.dram_tensor("msg1t", [MID, N], F32, kind="ExternalInput")
    zwo1_d = nc.dram_tensor("zwo1", [N, OUT], F32, kind="ExternalInput")
    zbc_d = nc.dram_tensor("zbc", [128, 2], F32, kind="ExternalInput")
    wme_d = nc.dram_tensor("wme", [H, MID], F32, kind="ExternalInput")
    wo2_d = nc.dram_tensor("wo2", [MID, OUT], F32, kind="ExternalInput")
    ident_d = nc.dram_tensor("ident", [128, 128], F32, kind="ExternalInput")
    out_d = nc.dram_tensor("out", [N, OUT], F32, kind="ExternalOutput")

    with tile.TileContext(nc) as tc:
        with (
            tc.tile_pool(name="const", bufs=1) as cpool,
            tc.tile_pool(name="eraw", bufs=4) as rpool,
            tc.tile_pool(name="edt", bufs=4) as epool,
            tc.tile_pool(name="xt", bufs=2, space="PSUM") as xpool,
            tc.tile_pool(name="grp", bufs=2, space="PSUM") as gpool,
        ):
            # ---- constants / tables ----
            ident_sb = cpool.tile([128, 128], F32)
            nc.sync.dma_start(out=ident_sb, in_=ident_d[:, :])
            wme_sb = cpool.tile([H, MID], F32)
            nc.sync.dma_start(out=wme_sb, in_=wme_d[:, :])
            wo2_sb = cpool.tile([MID, OUT], F32)
            nc.sync.dma_start(out=wo2_sb, in_=wo2_d[:, :])
            msg1t_sb = cpool.tile([MID, N], F32)
            nc.sync.dma_start(out=msg1t_sb, in_=msg1t_d[:, :])
            zwo1_sb = cpool.tile([128, 2, OUT], F32)
            nc.sync.dma_start(
                out=zwo1_sb, in_=zwo1_d.rearrange("(t p) m -> p t m", p=128)
            )
            zbc_sb = cpool.tile([128, 2], F32)
            nc.sync.dma_start(out=zbc_sb, in_=zbc_d[:, :])
            # lhsT pairs for the K=2 rank-1: partition 32g   = c rows
            #                                partition 32g+1 = ones
            ctab_sb = cpool.tile([128, 64 * MID], F32, padded_shape=None)
            nc.sync.dma_start(out=ctab_sb[0:128:32, :], in_=ctab_d[0:8:2, :])
            nc.sync.dma_start(out=ctab_sb[1:128:32, :], in_=ctab_d[1:8:2, :])
            # rhs pairs: partition 32g = ones, 32g+1 = M' rows
            mtab_sb = cpool.tile([128, 64 * N], F32)
            nc.sync.dma_start(out=mtab_sb[0:128:32, :], in_=mtab_d[0:8:2, :])
            nc.sync.dma_start(out=mtab_sb[1:128:32, :], in_=mtab_d[1:8:2, :])

            # ---- PE warmup: one tiny transpose per constant DMA so the PE
            # engine's observed semaphore clock covers every constant before
            # the main loop. Walrus allows only one embedded sync-wait on a
            # (fp32) Matmult, so in-loop matmuls must never need to wait on
            # these lanes. All warmups write overlapping scratch -> WAW chain
            # keeps them in order with exactly one new wait each.
            scratch = xpool.tile([128, 256], F32, name="scratch", tag="xt")
            warm_srcs = [
                (ident_sb, 128),
                (wme_sb, 128),
                (wo2_sb, 128),
                (msg1t_sb[:, 0:128], 128),
                (zwo1_sb[:, 0, :], 128),
                (zbc_sb, 128),
                (ctab_sb[0:1, 0:128], 1),
                (ctab_sb[0:2, 0:128], 2),
                (mtab_sb[0:1, 0:128], 1),
                (mtab_sb[0:2, 0:128], 2),
            ]
            last_warm = None
            for src, k in warm_srcs:
                m = src.free_size()
                last_warm = nc.tensor.transpose(
                    out=scratch[0:m, 0:k], in_=src, identity=ident_sb[0:k, 0:k]
                )

            # ---- main loop over source nodes i ----
            raw_tiles = {}

            def get_raw(i):
                ti = i // 8
                if ti not in raw_tiles:
                    rt = rpool.tile([128, 8, 2, H], F32, name=f"raw{ti}", tag="raw")
                    nc.sync.dma_start(
                        out=rt,
                        in_=edge[ti * 8 : (ti + 1) * 8].rearrange(
                            "i (t p) h -> p i t h", p=128
                        ),
                    )
                    raw_tiles[ti] = rt
                return raw_tiles[ti]

            prev_grp = None
            cur_grp = gpool.tile([128, 1536], F32, name="grp0", tag="grp")
            i0 = 0
            reduce_insts = []
            for gi, gsz in enumerate(GROUPS):
                # chain: reduce of the previous group -> this group's acc slot
                if prev_grp is not None:
                    psz = GROUPS[gi - 1]
                    if psz == 5:
                        cnt = 6 if gi >= 2 else 5
                        rin = prev_grp[:, 0 : cnt * 256].rearrange(
                            "p (c j) -> p j c", j=256
                        )
                    else:
                        rin = prev_grp[:, 0:1536].rearrange(
                            "p (c j) -> p j c", j=256
                        )[:, :, 0:6:5]
                    reduce_insts.append(
                        nc.vector.tensor_reduce(
                            out=cur_grp[:, 1280:1536],
                            in_=rin,
                            axis=mybir.AxisListType.X,
                            op=mybir.AluOpType.max,
                        )
                    )
                for di in range(gsz):
                    i = i0 + di
                    rt = get_raw(i)
                    il = i % 8
                    xt = xpool.tile([128, 256], F32, name=f"xt{i}", tag="xt")
                    t1 = nc.tensor.transpose(
                        out=xt[:, 0:128], in_=rt[:, il, 0, :], identity=ident_sb
                    )
                    if i == 0:
                        add_dep_helper(t1.ins, last_warm.ins, reason="pe warmup first")
                    nc.tensor.transpose(
                        out=xt[:, 128:256], in_=rt[:, il, 1, :], identity=ident_sb
                    )
                    edt = epool.tile([128, 256], F32, name=f"edt{i}", tag="edt")
                    cp = nc.scalar.copy(out=edt, in_=xt[:, 0:256])
                    if reduce_insts:
                        # route the PSUM-slot WAR / bank-sharing dependency
                        # through ACT so the consuming matmul needs only one
                        # wait (on ACT), never a second (on DVE).
                        add_dep_helper(
                            cp.ins, reduce_insts[-1].ins, reason="psum slot via act"
                        )
                    pl = cur_grp[:, di * 256 : (di + 1) * 256]
                    nc.tensor.matmul(
                        out=pl, lhsT=wme_sb, rhs=edt, start=True, stop=False
                    )
                    g, q = i % 4, i // 4
                    nc.tensor.matmul(
                        out=pl,
                        lhsT=ctab_sb[32 * g : 32 * g + 2, q * MID : (q + 1) * MID],
                        rhs=mtab_sb[32 * g : 32 * g + 2, q * N : (q + 1) * N],
                        start=False,
                        stop=True,
                        tile_position=(32 * g, 0),
                    )
                i0 += gsz
                prev_grp = cur_grp
                cur_grp = gpool.tile([128, 1536], F32, name=f"grp{gi + 1}", tag="grp")

            # final reduce: last group has 1 plane (P0 @ 0) + acc @ 1280
            rin = prev_grp[:, 0:1536].rearrange("p (c j) -> p j c", j=256)[
                :, :, 0:6:5
            ]
            facc = cur_grp[:, 1280:1536]
            nc.vector.tensor_reduce(
                out=facc, in_=rin, axis=mybir.AxisListType.X, op=mybir.AluOpType.max
            )

            # ---- epilogue ----
            a_sb = cpool.tile([MID, N], F32)
            nc.vector.tensor_tensor(
                out=a_sb, in0=facc, in1=msg1t_sb, op=mybir.AluOpType.add
            )
            xtf = xpool.tile([128, 256], F32, name="xtf", tag="xt")
            nc.tensor.transpose(out=xtf[:, 0:128], in_=a_sb[:, 0:128], identity=ident_sb)
            nc.tensor.transpose(
                out=xtf[:, 128:256], in_=a_sb[:, 128:256], identity=ident_sb
            )
            msgs_sb = cpool.tile([128, 2, MID], F32)
            nc.vector.tensor_scalar(
                out=msgs_sb[:, 0, :],
                in0=xtf[:, 0:128],
                scalar1=zbc_sb[:, 0:1],
                scalar2=None,
                op0=mybir.AluOpType.max,
            )
            nc.vector.tensor_scalar(
                out=msgs_sb[:, 1, :],
                in0=xtf[:, 128:256],
                scalar1=zbc_sb[:, 1:2],
                scalar2=None,
                op0=mybir.AluOpType.max,
            )
            xtg = xpool.tile([128, 256], F32, name="xtg", tag="xt")
            nc.tensor.transpose(
                out=xtg[:, 0:128], in_=msgs_sb[:, 0, :], identity=ident_sb
            )
            nc.tensor.transpose(
                out=xtg[:, 128:256], in_=msgs_sb[:, 1, :], identity=ident_sb
            )
            msgst_sb = cpool.tile([MID, N], F32)
            nc.scalar.copy(out=msgst_sb, in_=xtg[:, 0:256])
            out_ps = xpool.tile([128, 256], F32, name="out_ps", tag="xt")
            for t in range(2):
                sl = out_ps[:, t * 128 : (t + 1) * 128]
                nc.tensor.matmul(
                    out=sl,
                    lhsT=msgst_sb[:, t * 128 : (t + 1) * 128],
                    rhs=wo2_sb,
                    start=True,
                    stop=False,
                )
                nc.tensor.matmul(
                    out=sl, lhsT=ident_sb, rhs=zwo1_sb[:, t, :], start=False, stop=True
                )
            out_sb = cpool.tile([128, 2, OUT], F32)
            nc.scalar.copy(out=out_sb, in_=out_ps[:, 0:256])
            nc.sync.dma_start(
                out=out_d.rearrange("(t p) m -> p t m", p=128), in_=out_sb
            )
    nc.compile()
    return nc


_NC_CACHE = {}


def _get_nc():
    if "nc" not in _NC_CACHE:
        _NC_CACHE["nc"] = build_nc()
    return _NC_CACHE["nc"]


def _pack4(a, inner):
    # rows i=4q+g -> row g, cols [q*inner:(q+1)*inner]
    return (
        a.reshape(64, 4, inner).transpose(1, 0, 2).reshape(4, 64 * inner)
    )


def prepare_inputs(
    node_fts, edge_fts, graph_fts, adj_mat, hidden,
    W_m1, b_m1, W_m2, b_m2, W_me, b_me, W_mg, b_mg, W_o1, b_o1, W_o2, b_o2,
):
    f32 = np.float32
    z = np.concatenate([node_fts, hidden], axis=-1).astype(f32)  # (B, N, 2H)
    msg1t = (z @ W_m1 + b_m1).transpose(0, 2, 1)  # (B, MID, N)
    cvec = graph_fts @ W_mg + (b_m2 + b_me + b_mg)  # (B, MID)
    c = z @ W_m2 + cvec[:, None, :]  # (B, N, MID)
    mprime = (adj_mat.astype(f32) - 1.0) * BIG  # (B, N, N)
    anyzero = adj_mat.min(axis=1) == 0  # (B, N) per column j
    zb = np.where(anyzero, 0.0, NEG).astype(f32)
    zbc = zb.reshape(B, 2, 128).transpose(0, 2, 1)  # (B, 128, 2)
    zwo1 = z @ W_o1 + (b_o1 + b_o2)  # (B, N, OUT)

    ctab = np.empty((B, 8, 64 * MID), f32)
    mtab = np.empty((B, 8, 64 * N), f32)
    for b in range(B):
        ctab[b, 0::2] = _pack4(c[b].astype(f32), MID)
        ctab[b, 1::2] = 1.0
        mtab[b, 0::2] = 1.0
        mtab[b, 1::2] = _pack4(mprime[b], N)

    ident = np.eye(128, dtype=f32)
    in_maps = []
    for b in range(B):
        in_maps.append(
            {
                "edge": np.ascontiguousarray(edge_fts[b], dtype=f32),
                "ctab": ctab[b],
                "mtab": mtab[b],
                "msg1t": np.ascontiguousarray(msg1t[b], dtype=f32),
                "zwo1": np.ascontiguousarray(zwo1[b], dtype=f32),
                "zbc": np.ascontiguousarray(zbc[b], dtype=f32),
                "wme": np.asarray(W_me, dtype=f32),
                "wo2": np.asarray(W_o2, dtype=f32),
                "ident": ident,
            }
        )
    return in_maps


def kernel(**inputs):
    inputs = {k: np.asarray(v) for k, v in inputs.items()}
    in_maps = prepare_inputs(**inputs)
    nc = _get_nc()
    res = run_bass_kernel_spmd(nc, in_maps, list(range(B)))
    return np.stack([np.asarray(res.results[b]["out"]) for b in range(B)])


if __name__ == "__main__":
    rng = np.random.default_rng(0)
    print("smoke build only")
    build_nc()
    print("build ok")


# revision 8
# speedup vs baseline: 1.0131x; 1.0131x over previous
"""Trainium2 Bass kernel for GNN message passing (nn_FALR2_35794257445089).

Math (per batch element b, per-core shapes):
    z = concat(node_fts, hidden)                       (n, 2h)
    msgs[i, j, m] = msg1[j,m] + msg2[i,m] + msgE[i,j,m] + msgG[m]
    out_msgs[j, m] = max_i msgs[i,j,m] * adj[i,j]
    ret = z @ W_o1 + b_o1 + out_msgs @ W_o2 + b_o2

Strategy: data-parallel over b across 8 cores; each core streams its own
32 MiB edge_fts[b] slice once (memory-bound regime).

Two host-side foldings make the device inner loop a pure
transpose->matmul->max pipeline:

1. Additive masking: with M'[i,j] = (adj[i,j]-1)*1e9, the candidates
   become msgE + c + M' (c = msg2 + msgG + biases), the i-independent
   msg1 pulls out of the max, and the reference's "masked entries
   contribute 0" / "all-masked columns are 0" semantics are restored by
   a final per-column clamp max(. , zb[j]).

2. The additive terms are folded into the edge data itself:
       edge_aug[i,j,:] = edge[i,j,:] + W_me^-T c[i,:] + M'[i,j] * v,
   v = W_me^-T 1, so that edge_aug @ W_me = msgE + c + M' and the device
   never touches c or the mask. (Masked entries only need to be hugely
   negative, so the fp error of that cancellation is irrelevant;
   unmasked entries see ~1e-4 absolute error from folded-constant
   rounding.)

Device inner loop (j = target node, 256 iters, "j-outer"):
  DMA : contiguous 1 MiB loads (128 i-partitions x 8 KiB lines)
  PE  : 2 transposes of edge_aug[:,j,:] (i,h)->(h,i) + 1 matmul
        W_me^T @ edgeT (N=256) into a PSUM plane (m, i)
  ACT : copy transposed-edge PSUM->SBUF (matmul input)
  DVE : one grouped tensor_reduce(max) over 6 planes along i,
        writing acc[:, j0:j0+6] columns directly (no merge step)
Epilogue: A = acc + msg1T; transpose, clamp vs zb, transpose back;
ret = msgsT^T @ W_o2 + (z @ W_o1 + biases) via identity-add matmul.
"""

import os
import sys

import numpy as np

if "/opt/trn_rl_repo" not in sys.path:
    sys.path.insert(0, "/opt/trn_rl_repo")

import concourse.bass as bass
import concourse.bacc as bacc
import concourse.mybir as mybir
import concourse.tile as tile
from concourse.tile import add_dep_helper
from concourse.bass_utils import run_bass_kernel_spmd

B, N, H, MID, OUT = 8, 256, 128, 128, 128
F32 = mybir.dt.float32
BIG = 1.0e9
NEG = -1.0e30

GROUPS = [6] * 42 + [4]  # 256 target nodes j
assert sum(GROUPS) == N


def build_nc():
    nc = bacc.Bacc("TRN2", target_bir_lowering=False, debug=False)

    edge = nc.dram_tensor("edge", [N, N, H], F32, kind="ExternalInput")
    msg1t_d = nc.dram_tensor("msg1t", [MID, N], F32, kind="ExternalInput")
    zwo1_d = nc.dram_tensor("zwo1", [N, OUT], F32, kind="ExternalInput")
    zbc_d = nc.dram_tensor("zbc", [128, 2], F32, kind="ExternalInput")
    wme_d = nc.dram_tensor("wme", [H, MID], F32, kind="ExternalInput")
    wo2_d = nc.dram_tensor("wo2", [MID, OUT], F32, kind="ExternalInput")
    ident_d = nc.dram_tensor("ident", [128, 128], F32, kind="ExternalInput")
    out_d = nc.dram_tensor("out", [N, OUT], F32, kind="ExternalOutput")

    with tile.TileContext(nc) as tc:
        with (
            tc.tile_pool(name="const", bufs=1) as cpool,
            tc.tile_pool(name="eraw", bufs=6) as rpool,
            tc.tile_pool(name="edt", bufs=6) as epool,
            tc.tile_pool(name="xt", bufs=2, space="PSUM") as xpool,
            tc.tile_pool(name="grp", bufs=2, space="PSUM") as gpool,
        ):
            # ---- constants ----
            ident_sb = cpool.tile([128, 128], F32)
            nc.sync.dma_start(out=ident_sb, in_=ident_d[:, :])
            wme_sb = cpool.tile([H, MID], F32)
            nc.sync.dma_start(out=wme_sb, in_=wme_d[:, :])
            wo2_sb = cpool.tile([MID, OUT], F32)
            nc.sync.dma_start(out=wo2_sb, in_=wo2_d[:, :])
            msg1t_sb = cpool.tile([MID, N], F32)
            nc.sync.dma_start(out=msg1t_sb, in_=msg1t_d[:, :])
            zwo1_sb = cpool.tile([128, 2, OUT], F32)
            nc.sync.dma_start(
                out=zwo1_sb, in_=zwo1_d.rearrange("(t p) m -> p t m", p=128)
            )
            zbc_sb = cpool.tile([128, 2], F32)
            nc.sync.dma_start(out=zbc_sb, in_=zbc_d[:, :])
            acc_sb = cpool.tile([MID, N], F32)

            # ---- PE warmup: cover every constant-DMA semaphore on the PE
            # clock so in-loop matmuls never need a second embedded wait.
            scratch = xpool.tile([128, 256], F32, name="scratch", tag="xt")
            warm_srcs = [
                ident_sb,
                wme_sb,
                wo2_sb,
                msg1t_sb[:, 0:128],
                zwo1_sb[:, 0, :],
                zbc_sb,
            ]
            last_warm = None
            for src in warm_srcs:
                m = src.free_size()
                last_warm = nc.tensor.transpose(
                    out=scratch[0:m, 0:128], in_=src, identity=ident_sb
                )

            # ---- main loop over target nodes j ----
            raw_tiles = {}

            def get_raw(jc, ih):
                key = (jc, ih)
                if key not in raw_tiles:
                    rt = rpool.tile(
                        [128, 16, H], F32, name=f"raw{jc}_{ih}", tag="raw"
                    )
                    nc.sync.dma_start(
                        out=rt,
                        in_=edge[
                            ih * 128 : (ih + 1) * 128, jc * 16 : (jc + 1) * 16, :
                        ],
                    )
                    raw_tiles[key] = rt
                return raw_tiles[key]

            reduce_insts = []
            j0 = 0
            for gi, gsz in enumerate(GROUPS):
                grp = gpool.tile([128, 1536], F32, name=f"grp{gi}", tag="grp")
                for dj in range(gsz):
                    j = j0 + dj
                    jc, jl = j // 16, j % 16
                    r0 = get_raw(jc, 0)
                    r1 = get_raw(jc, 1)
                    xt = xpool.tile([128, 256], F32, name=f"xt{j}", tag="xt")
                    t1 = nc.tensor.transpose(
                        out=xt[:, 0:128], in_=r0[:, jl, :], identity=ident_sb
                    )
                    if j == 0:
                        add_dep_helper(
                            t1.ins, last_warm.ins, reason="pe warmup first"
                        )
                    nc.tensor.transpose(
                        out=xt[:, 128:256], in_=r1[:, jl, :], identity=ident_sb
                    )
                    edt = epool.tile([128, 256], F32, name=f"edt{j}", tag="edt")
                    cp = nc.scalar.copy(out=edt, in_=xt[:, 0:256])
                    if reduce_insts:
                        # route the grp-slot WAR dependency through ACT so the
                        # matmul below needs only its single ACT wait.
                        add_dep_helper(
                            cp.ins,
                            reduce_insts[-1].ins,
                            reason="psum slot via act",
                        )
                    nc.tensor.matmul(
                        out=grp[:, dj * 256 : (dj + 1) * 256],
                        lhsT=wme_sb,
                        rhs=edt,
                        start=True,
                        stop=True,
                    )
                rin = grp[:, 0 : gsz * 256].rearrange("p (c i) -> p c i", i=256)
                reduce_insts.append(
                    nc.vector.tensor_reduce(
                        out=acc_sb[:, j0 : j0 + gsz],
                        in_=rin,
                        axis=mybir.AxisListType.X,
                        op=mybir.AluOpType.max,
                    )
                )
                j0 += gsz

            # ---- epilogue ----
            a_sb = cpool.tile([MID, N], F32)
            nc.vector.tensor_tensor(
                out=a_sb, in0=acc_sb, in1=msg1t_sb, op=mybir.AluOpType.add
            )
            xtf = xpool.tile([128, 256], F32, name="xtf", tag="xt")
            nc.tensor.transpose(
                out=xtf[:, 0:128], in_=a_sb[:, 0:128], identity=ident_sb
            )
            nc.tensor.transpose(
                out=xtf[:, 128:256], in_=a_sb[:, 128:256], identity=ident_sb
            )
            msgs_sb = cpool.tile([128, 2, MID], F32)
            nc.vector.tensor_scalar(
                out=msgs_sb[:, 0, :],
                in0=xtf[:, 0:128],
                scalar1=zbc_sb[:, 0:1],
                scalar2=None,
                op0=mybir.AluOpType.max,
            )
            nc.vector.tensor_scalar(
                out=msgs_sb[:, 1, :],
                in0=xtf[:, 128:256],
                scalar1=zbc_sb[:, 1:2],
                scalar2=None,
                op0=mybir.AluOpType.max,
            )
            xtg = xpool.tile([128, 256], F32, name="xtg", tag="xt")
            nc.tensor.transpose(
                out=xtg[:, 0:128], in_=msgs_sb[:, 0, :], identity=ident_sb
            )
            nc.tensor.transpose(
                out=xtg[:, 128:256], in_=msgs_sb[:, 1, :], identity=ident_sb
            )
            msgst_sb = cpool.tile([MID, N], F32)
            nc.scalar.copy(out=msgst_sb, in_=xtg[:, 0:256])
            out_ps = xpool.tile([128, 256], F32, name="out_ps", tag="xt")
            for t in range(2):
                sl = out_ps[:, t * 128 : (t + 1) * 128]
                nc.tensor.matmul(
                    out=sl,
                    lhsT=msgst_sb[:, t * 128 : (t + 1) * 128],
                    rhs=wo2_sb,
                    start=True,
                    stop=False,
                )
                nc.tensor.matmul(
                    out=sl,
                    lhsT=ident_sb,
                    rhs=zwo1_sb[:, t, :],
                    start=False,
                    stop=True,
                )
            out_sb = cpool.tile([128, 2, OUT], F32)
            nc.scalar.copy(out=out_sb, in_=out_ps[:, 0:256])
            nc.sync.dma_start(
                out=out_d.rearrange("(t p) m -> p t m", p=128), in_=out_sb
            )
    nc.compile()
    return nc


_NC_CACHE = {}


def _get_nc():
    if "nc" not in _NC_CACHE:
        _NC_CACHE["nc"] = build_nc()
    return _NC_CACHE["nc"]


def prepare_inputs(
    node_fts, edge_fts, graph_fts, adj_mat, hidden,
    W_m1, b_m1, W_m2, b_m2, W_me, b_me, W_mg, b_mg, W_o1, b_o1, W_o2, b_o2,
):
    f32 = np.float32
    z = np.concatenate([node_fts, hidden], axis=-1).astype(f32)  # (B, N, 2H)
    msg1t = (z @ W_m1 + b_m1).transpose(0, 2, 1)  # (B, MID, N)
    cvec = graph_fts @ W_mg + (b_m2 + b_me + b_mg)  # (B, MID)
    c = z @ W_m2 + cvec[:, None, :]  # (B, N, MID) indexed by source i

    # fold c and the additive mask into the edge data:
    #   edge_aug @ W_me = msgE + c + (adj-1)*BIG
    WmeT = np.asarray(W_me, dtype=np.float64).T  # (MID, H) acting on h
    e_extra = np.linalg.solve(
        WmeT, c.astype(np.float64).transpose(0, 2, 1)
    ).transpose(0, 2, 1)  # (B, N, H)
    v = np.linalg.solve(WmeT, np.ones(MID))  # (H,)
    mprime = (adj_mat.astype(np.float64) - 1.0) * BIG  # (B, N, N)
    edge_aug = (
        edge_fts.astype(np.float64)
        + e_extra[:, :, None, :]
        + mprime[..., None] * v
    ).astype(f32)

    anyzero = adj_mat.min(axis=1) == 0  # (B, N) per target column j
    zb = np.where(anyzero, 0.0, NEG).astype(f32)
    zbc = zb.reshape(B, 2, 128).transpose(0, 2, 1)  # (B, 128, 2)
    zwo1 = z @ W_o1 + (b_o1 + b_o2)  # (B, N, OUT)

    ident = np.eye(128, dtype=f32)
    in_maps = []
    for b in range(B):
        in_maps.append(
            {
                "edge": np.ascontiguousarray(edge_aug[b]),
                "msg1t": np.ascontiguousarray(msg1t[b], dtype=f32),
                "zwo1": np.ascontiguousarray(zwo1[b], dtype=f32),
                "zbc": np.ascontiguousarray(zbc[b], dtype=f32),
                "wme": np.asarray(W_me, dtype=f32),
                "wo2": np.asarray(W_o2, dtype=f32),
                "ident": ident,
            }
        )
    return in_maps


def kernel(**inputs):
    inputs = {k: np.asarray(v) for k, v in inputs.items()}
    in_maps = prepare_inputs(**inputs)
    nc = _get_nc()
    res = run_bass_kernel_spmd(nc, in_maps, list(range(B)))
    return np.stack([np.asarray(res.results[b]["out"]) for b in range(B)])


if __name__ == "__main__":
    print("smoke build only")
    build_nc()
    print("build ok")
